# revision 12
# baseline (speedup 1.0000x reference)
"""Distributed Trainium2 kernel for relative-position-bias multi-head attention.

Problem: B=1, L=4096, D=512, H=8, HD=64.
    x = x + pos_embed
    q,k,v = x @ W{q,k,v} + b{q,k,v}   (per head)
    scores = (q/8) @ k^T + rel_bias_toeplitz
    out = softmax(scores) @ v ; out = out @ Wo + bo

Sharding: head-parallel. Core h owns head h.
  1. Each core transposes+adds its L/8 chunk of (x, pos) -> xp^T chunk [D, 512],
     AllGather -> full xp^T [D, L] on every core.
  2. Q^T,K^T [64, L] and token-major V [L, 64] for head h (f32r matmuls).
  3. Flash over score tiles in TRANSPOSED layout scores^T [k-part 128, q-free 512]:
     matmul(K^T-slice as weights, Q^T as moving) -> +staircase bias (DVE) ->
     exp (ACT, no max-subtraction; scores ~N(0,1)) -> accumulate
     O^T_unnorm [65, q] via matmul with augmented weights [V | ones].
     Row 64 = softmax denominator. Normalize via reciprocal + ones-outer-product
     partition replication.
  4. AllToAll redistributes head outputs to sequence-parallel rows; each core
     computes its own 512 rows of the output projection. Host transposes and
     concatenates the per-core [D, 512] outputs.

The rel-bias Toeplitz staircase (bias[i,j] = rel[h, L-1+j-i]) is materialized
host-side as one [128, 8064] array per head; every (k-block, q-chunk) bias tile
is a plain column slice of it.
"""
import sys
sys.path.insert(0, '/opt/trn_rl_repo')
import dataclasses

import numpy as np

import concourse.bass as bass
import concourse.tile as tile
from concourse import bacc, mybir

B, L, D, H = 1, 4096, 512, 8
HD = D // H            # 64
NCORES = 8
LC = L // NCORES       # 512 sequence rows per core
NDCH = D // 128        # 4 contraction chunks
QW = 512               # q-chunk width (free dim of score tiles)
NQ = L // QW           # 8
KB = 128               # k-block (partition dim of score tiles)
NK = L // KB           # 32
SW = 8064              # staircase width: col c0 = 3968 + q0 - k0, + 512
F32 = mybir.dt.float32
F32R = mybir.dt.float32r
BF16 = mybir.dt.bfloat16


def _r(ap, offset, pattern):
    return dataclasses.replace(ap, offset=offset, ap=pattern)


def build():
    nc = bacc.Bacc(None, target_bir_lowering=False)

    xT = nc.declare_dram_parameter("xT", [D, LC], F32, isOutput=False)
    posT = nc.declare_dram_parameter("posT", [D, LC], F32, isOutput=False)
    stair = nc.declare_dram_parameter("stair", [128, SW], BF16, isOutput=False)
    wq = nc.declare_dram_parameter("wq", [D, HD], F32R, isOutput=False)
    wk = nc.declare_dram_parameter("wk", [D, HD], F32R, isOutput=False)
    wv = nc.declare_dram_parameter("wv", [D, HD], F32R, isOutput=False)
    bq = nc.declare_dram_parameter("bq", [HD, 1], F32, isOutput=False)
    bk = nc.declare_dram_parameter("bk", [HD, 1], F32, isOutput=False)
    bvr = nc.declare_dram_parameter("bvr", [128, HD], F32, isOutput=False)
    wo = nc.declare_dram_parameter("wo", [HD, D], F32R, isOutput=False)
    bo = nc.declare_dram_parameter("bo", [D, 1], F32, isOutput=False)
    out = nc.declare_dram_parameter("out", [D, LC], F32, isOutput=True)

    rg = [list(range(NCORES))]
    Exp = mybir.ActivationFunctionType.Exp

    with tile.TileContext(nc) as tc:
        with (
            nc.allow_low_precision(reason="fp32r matmuls; tolerance 2e-2"),
            tc.tile_pool(name="const", bufs=1) as constp,
            tc.tile_pool(name="xin", bufs=1) as xin,
            tc.tile_pool(name="proj", bufs=1) as projp,
            tc.tile_pool(name="ps_pj", bufs=2, space="PSUM") as ps_pj,
            tc.tile_pool(name="ps_s", bufs=2, space="PSUM") as ps_sp,
            tc.tile_pool(name="ps_o", bufs=2, space="PSUM") as ps_op,
            tc.tile_pool(name="ps_r", bufs=1, space="PSUM") as ps_rp,
            tc.tile_pool(name="attn", bufs=3) as attnp,
            tc.tile_pool(name="work", bufs=2) as workp,
            tc.tile_pool(name="dram", bufs=1, space="DRAM") as dram,
        ):
            # ---------------- constants / weights into SBUF ----------------
            stair_sb = constp.tile([128, SW], BF16)
            nc.sync.dma_start(stair_sb[:], stair[:, :])

            # w{q,k,v} [D, HD] -> [128, NDCH*HD]; chunk c in cols [HD*c, HD*(c+1))
            wsbs = {}
            for name, w in (("q", wq), ("k", wk), ("v", wv)):
                t = constp.tile([128, NDCH * HD], F32R)
                nc.sync.dma_start(
                    t[:], _r(w.ap(), 0, [[HD, 128], [128 * HD, NDCH], [1, HD]])
                )
                wsbs[name] = t
            # wo (head h block) [hd, Dd] -> [64, D]; lhsT slice [:, 128*pd : ...]
            wo_sb = constp.tile([HD, D], F32R)
            nc.sync.dma_start(wo_sb[:], wo[:, :])
            bq_sb = constp.tile([HD, 1], F32)
            nc.sync.dma_start(bq_sb[:], bq[:, :])
            bk_sb = constp.tile([HD, 1], F32)
            nc.sync.dma_start(bk_sb[:], bk[:, :])
            bvr_sb = constp.tile([128, HD], F32)
            nc.sync.dma_start(bvr_sb[:], bvr[:, :])
            bo_sb = constp.tile([128, NDCH], F32)  # chunk pd in col pd
            nc.sync.dma_start(
                bo_sb[:], _r(bo.ap(), 0, [[1, 128], [128, NDCH]])
            )
            ones_f32 = constp.tile([1, HD], F32)
            nc.vector.memset(ones_f32[:], 1.0)
            ones_sb = constp.tile([1, HD], F32R)
            nc.vector.tensor_copy(ones_sb[:], ones_f32[:])

            # ---------------- xp^T chunk + AllGather ----------------
            ag_in = dram.tile([D, LC], F32R)
            for c in range(NDCH):
                t1 = xin.tile([128, LC], F32, tag="xt")
                nc.sync.dma_start(t1[:], xT[128 * c : 128 * (c + 1), :])
                t2 = xin.tile([128, LC], F32, tag="pos")
                nc.sync.dma_start(t2[:], posT[128 * c : 128 * (c + 1), :])
                xp = xin.tile([128, LC], F32R, tag="xp")
                nc.vector.tensor_add(xp[:], t1[:], t2[:])
                nc.sync.dma_start(ag_in[128 * c : 128 * (c + 1), :], xp[:])
            ag_out = dram.tile([NCORES * D, LC], F32R, addr_space="Shared")
            nc.gpsimd.collective_compute(
                "AllGather", mybir.AluOpType.bypass, replica_groups=rg,
                ins=[ag_in.opt()], outs=[ag_out.opt()],
            )
            # gathered xp^T -> 4 SBUF tiles [128, L] (d-chunk major, l = (rank, l'))
            xpT = []
            for c in range(NDCH):
                t = xin.tile([128, L], F32R, tag=f"xpT{c}")
                base = ag_out[:]
                nc.sync.dma_start(
                    t[:],
                    _r(base, base.offset + c * 128 * LC,
                       [[LC, 128], [D * LC, NCORES], [1, LC]]),
                )
                xpT.append(t)

            # ---------------- projections ----------------
            qT = projp.tile([HD, L], F32R, tag="qT")
            kT = projp.tile([HD, L], F32R, tag="kT")
            for wname, bias_sb, dst in (("q", bq_sb, qT), ("k", bk_sb, kT)):
                wt = wsbs[wname]
                for n in range(L // 512):
                    ps = ps_pj.tile([HD, 512], F32, tag="pj")
                    for c in range(NDCH):
                        nc.tensor.matmul(
                            ps[:],
                            wt[:, HD * c : HD * (c + 1)],
                            xpT[c][:, 512 * n : 512 * (n + 1)],
                            start=(c == 0), stop=(c == NDCH - 1),
                        )
                    nc.vector.tensor_scalar_add(
                        dst[:, 512 * n : 512 * (n + 1)], ps[:], bias_sb[:]
                    )
            # token-major V, augmented with a ones column -> [128, 65] per k-block
            vaug = constp.tile([128, 65 * NK], BF16)
            nc.vector.memset(vaug[:, HD::65], 1.0)
            wt = wsbs["v"]
            for lb in range(NK):
                psv = ps_pj.tile([128, HD], F32, tag="pj")
                for c in range(NDCH):
                    nc.tensor.matmul(
                        psv[:],
                        xpT[c][:, 128 * lb : 128 * (lb + 1)],
                        wt[:, HD * c : HD * (c + 1)],
                        start=(c == 0), stop=(c == NDCH - 1),
                    )
                nc.vector.tensor_add(
                    vaug[:, 65 * lb : 65 * lb + HD], psv[:], bvr_sb[:]
                )

            # ---------------- flash attention (transposed layout) ----------------
            oT = projp.tile([HD, L], F32R, tag="oT")  # normalized head output
            rs_in = dram.tile([NCORES, D, QW], F32)
            rs_out = dram.tile([D, QW], F32)
            for qc in range(NQ):
                q0 = qc * QW
                pso = ps_op.tile([HD + 1, QW], F32, tag="Oacc")
                for kb in range(NK):
                    k0 = kb * KB
                    pss = ps_sp.tile([KB, QW], F32, tag="s")
                    nc.tensor.matmul(
                        pss[:],
                        kT[:, k0 : k0 + KB],
                        qT[:, q0 : q0 + QW],
                        start=True, stop=True,
                    )
                    st = attnp.tile([KB, QW], BF16, tag="st")
                    nc.scalar.activation(st[:], pss[:], Exp)
                    at = attnp.tile([KB, QW], BF16, tag="at")
                    c0 = 3968 + q0 - k0
                    nc.vector.tensor_mul(
                        at[:], st[:], stair_sb[:, c0 : c0 + QW]
                    )
                    nc.tensor.matmul(
                        pso[:],
                        vaug[:, 65 * kb : 65 * (kb + 1)],
                        at[:],
                        start=(kb == 0), stop=(kb == NK - 1),
                    )
                # normalize: rows 0..63 / row 64
                rec = workp.tile([1, QW], F32R, tag="rec")
                nc.vector.reciprocal(rec[:], pso[HD : HD + 1, :])
                psr = ps_rp.tile([HD, QW], F32, tag="rep")
                nc.tensor.matmul(
                    psr[:], ones_sb[:], rec[:],
                    start=True, stop=True,
                )
                rep = workp.tile([HD, QW], F32, tag="rep_sb")
                nc.vector.tensor_copy(rep[:], psr[:])
                nc.vector.tensor_mul(oT[:, q0 : q0 + QW], pso[0:HD, :], rep[:])
                # fused partial output projection for this q-chunk
                for pd in range(NDCH):
                    psw = ps_rp.tile([128, QW], F32, tag="wo_ps")
                    nc.tensor.matmul(
                        psw[:], wo_sb[:, 128 * pd : 128 * (pd + 1)],
                        oT[:, q0 : q0 + QW],
                        start=True, stop=True,
                    )
                    wt_sb = workp.tile([128, QW], F32, tag="wo_sb_t")
                    nc.vector.tensor_copy(wt_sb[:], psw[:])
                    base = rs_in[:]
                    nc.sync.dma_start(
                        _r(base, base.offset + (qc * D + 128 * pd) * QW,
                           [[QW, 128], [1, QW]]),
                        wt_sb[:],
                    )

            # ---------------- ReduceScatter + bias ----------------
            nc.gpsimd.collective_compute(
                "ReduceScatter", mybir.AluOpType.add, replica_groups=rg,
                ins=[rs_in.opt()], outs=[rs_out.opt()],
            )
            for pd in range(NDCH):
                t = workp.tile([128, LC], F32, tag="rs_sb")
                nc.sync.dma_start(t[:], rs_out[128 * pd : 128 * (pd + 1), :])
                ot = workp.tile([128, LC], F32, tag="ot")
                nc.vector.tensor_scalar_add(ot[:], t[:], bo_sb[:, pd : pd + 1])
                nc.sync.dma_start(out[128 * pd : 128 * (pd + 1), :], ot[:])
    return nc


def make_in_maps(x, pos_embed, rel_bias, Wq, bq, Wk, bk, Wv, bv, Wo, bo):
    """Host-side sharding: returns per-core input dicts."""
    x = np.asarray(x, np.float32)
    pos = np.asarray(pos_embed, np.float32)
    rel = np.asarray(rel_bias, np.float32)
    import ml_dtypes
    # exp-staircase per head: stair[p, c] = exp(rel[h, 8063 + p - c]) in bf16
    idx = 8063 + np.arange(128)[:, None] - np.arange(SW)[None, :]
    in_maps = []
    for h in range(NCORES):
        chunk = slice(LC * h, LC * (h + 1))
        in_maps.append({
            "xT": np.ascontiguousarray(x[0, chunk, :].T),
            "posT": np.ascontiguousarray(pos[chunk, :].T),
            "stair": np.ascontiguousarray(np.exp(rel[h][idx])).astype(ml_dtypes.bfloat16),
            "wq": np.ascontiguousarray(Wq[:, h, :] / 8.0),
            "wk": np.ascontiguousarray(Wk[:, h, :]),
            "wv": np.ascontiguousarray(Wv[:, h, :]),
            "bq": np.ascontiguousarray(bq[h][:, None] / 8.0),
            "bk": np.ascontiguousarray(bk[h][:, None]),
            "bvr": np.ascontiguousarray(np.broadcast_to(bv[h], (128, HD))),
            "wo": np.ascontiguousarray(Wo[h]),
            "bo": np.ascontiguousarray(bo[:, None]),
        })
    return in_maps


_CACHE = {}


def _get_runner():
    """Build + finalize once; return a cached callable in_maps -> results."""
    if "run" in _CACHE:
        return _CACHE["run"]
    nc = build()
    nc.finalize()
    from concourse import bass_utils

    def run(in_maps):
        return bass_utils.run_bass_kernel_spmd(
            nc, in_maps, core_ids=list(range(NCORES))
        ).results

    _CACHE["run"] = run
    return run


def kernel(x, pos_embed, rel_bias, Wq, bq, Wk, bk, Wv, bv, Wo, bo):
    in_maps = make_in_maps(x, pos_embed, rel_bias, Wq, bq, Wk, bk, Wv, bv, Wo, bo)
    results = _get_runner()(in_maps)
    y = np.empty((B, L, D), np.float32)
    for c in range(NCORES):
        y[0, LC * c : LC * (c + 1), :] = results[c]["out"].T
    return y


# revision 15
# speedup vs baseline: 2.1268x; 2.1268x over previous
"""Distributed Trainium2 kernel for relative-position-bias multi-head attention.

Problem: B=1, L=4096, D=512, H=8, HD=64.
    x = x + pos_embed
    q,k,v = x @ W{q,k,v} + b{q,k,v}   (per head)
    scores = (q/8) @ k^T + rel_bias_toeplitz
    out = softmax(scores) @ v ; out = out @ Wo + bo

Sharding: head-parallel. Core h owns head h.
  1. Each core transposes+adds its L/8 chunk of (x, pos) -> xp^T chunk [D, 512],
     AllGather -> full xp^T [D, L] on every core.
  2. Q^T,K^T [64, L] and token-major V [L, 64] for head h (f32r matmuls).
  3. Flash over score tiles in TRANSPOSED layout scores^T [k-part 128, q-free 512]:
     matmul(K^T-slice as weights, Q^T as moving) -> +staircase bias (DVE) ->
     exp (ACT, no max-subtraction; scores ~N(0,1)) -> accumulate
     O^T_unnorm [65, q] via matmul with augmented weights [V | ones].
     Row 64 = softmax denominator. Normalize via reciprocal + ones-outer-product
     partition replication.
  4. AllToAll redistributes head outputs to sequence-parallel rows; each core
     computes its own 512 rows of the output projection. Host transposes and
     concatenates the per-core [D, 512] outputs.

The rel-bias Toeplitz staircase (bias[i,j] = rel[h, L-1+j-i]) is materialized
host-side as one [128, 8064] array per head; every (k-block, q-chunk) bias tile
is a plain column slice of it.
"""
import sys
sys.path.insert(0, '/opt/trn_rl_repo')
import dataclasses

import numpy as np

import concourse.bass as bass
import concourse.tile as tile
from concourse import bacc, mybir

B, L, D, H = 1, 4096, 512, 8
HD = D // H            # 64
NCORES = 8
LC = L // NCORES       # 512 sequence rows per core
NDCH = D // 128        # 4 contraction chunks
QW = 512               # q-chunk width (free dim of score tiles)
NQ = L // QW           # 8
KB = 128               # k-block (partition dim of score tiles)
NK = L // KB           # 32
SW = 8064              # staircase width: col c0 = 3968 + q0 - k0, + 512
AG_SHARED = True       # probes flip this when stubbing the AllGather
F32 = mybir.dt.float32
F32R = mybir.dt.float32r
BF16 = mybir.dt.bfloat16


def _r(ap, offset, pattern):
    return dataclasses.replace(ap, offset=offset, ap=pattern)


def build():
    nc = bacc.Bacc(None, target_bir_lowering=False)

    xT = nc.declare_dram_parameter("xT", [D, LC], BF16, isOutput=False)
    posT = nc.declare_dram_parameter("posT", [D, LC], BF16, isOutput=False)
    stair = nc.declare_dram_parameter("stair", [128, SW], BF16, isOutput=False)
    wq = nc.declare_dram_parameter("wq", [D, HD], BF16, isOutput=False)
    wk = nc.declare_dram_parameter("wk", [D, HD], BF16, isOutput=False)
    wv = nc.declare_dram_parameter("wv", [D, HD], BF16, isOutput=False)
    bq = nc.declare_dram_parameter("bq", [HD, 1], F32, isOutput=False)
    bk = nc.declare_dram_parameter("bk", [HD, 1], F32, isOutput=False)
    bvr = nc.declare_dram_parameter("bvr", [128, HD], F32, isOutput=False)
    wo = nc.declare_dram_parameter("wo", [HD, D], F32R, isOutput=False)
    bo = nc.declare_dram_parameter("bo", [D, 1], F32, isOutput=False)
    out = nc.declare_dram_parameter("out", [D, LC], F32, isOutput=True)

    rg = [list(range(NCORES))]
    Exp = mybir.ActivationFunctionType.Exp

    with tile.TileContext(nc) as tc:
        with (
            nc.allow_low_precision(reason="fp32r matmuls; tolerance 2e-2"),
            tc.tile_pool(name="const", bufs=1) as constp,
            tc.tile_pool(name="xin", bufs=1) as xin,
            tc.tile_pool(name="proj", bufs=1) as projp,
            tc.tile_pool(name="ps_pj", bufs=2, space="PSUM") as ps_pj,
            tc.tile_pool(name="ps_s", bufs=2, space="PSUM") as ps_sp,
            tc.tile_pool(name="ps_o", bufs=2, space="PSUM") as ps_op,
            tc.tile_pool(name="ps_r", bufs=1, space="PSUM") as ps_rp,
            tc.tile_pool(name="attn", bufs=3) as attnp,
            tc.tile_pool(name="work", bufs=2) as workp,
            tc.tile_pool(name="dram", bufs=1, space="DRAM") as dram,
        ):
            # ---------------- constants / weights into SBUF ----------------
            # w{q,k,v} [D, HD] -> [128, NDCH*HD]; chunk c in cols [HD*c, HD*(c+1))
            wsbs = {}
            for name, w in (("q", wq), ("k", wk), ("v", wv)):
                t = constp.tile([128, NDCH * HD], BF16)
                nc.sync.dma_start(
                    t[:], _r(w.ap(), 0, [[HD, 128], [128 * HD, NDCH], [1, HD]])
                )
                wsbs[name] = t
            # wo (head h block) [hd, Dd] -> [64, D]; lhsT slice [:, 128*pd : ...]
            wo_sb = constp.tile([HD, D], F32R)
            nc.sync.dma_start(wo_sb[:], wo[:, :])
            bq_sb = constp.tile([HD, 1], F32)
            nc.sync.dma_start(bq_sb[:], bq[:, :])
            bk_sb = constp.tile([HD, 1], F32)
            nc.sync.dma_start(bk_sb[:], bk[:, :])
            bvr_sb = constp.tile([128, HD], F32)
            nc.sync.dma_start(bvr_sb[:], bvr[:, :])
            bo_sb = constp.tile([128, NDCH], F32)  # chunk pd in col pd
            nc.sync.dma_start(
                bo_sb[:], _r(bo.ap(), 0, [[1, 128], [128, NDCH]])
            )
            ones_f32 = constp.tile([1, HD], F32)
            nc.vector.memset(ones_f32[:], 1.0)
            ones_sb = constp.tile([1, HD], F32R)
            nc.vector.tensor_copy(ones_sb[:], ones_f32[:])

            # ---------------- xp^T chunk + AllGather ----------------
            ag_in = dram.tile([D, LC], BF16)
            for c in range(NDCH):
                t1 = xin.tile([128, LC], BF16, tag="xt")
                nc.sync.dma_start(t1[:], xT[128 * c : 128 * (c + 1), :])
                t2 = xin.tile([128, LC], BF16, tag="pos")
                nc.sync.dma_start(t2[:], posT[128 * c : 128 * (c + 1), :])
                xp = xin.tile([128, LC], BF16, tag="xp")
                nc.vector.tensor_add(xp[:], t1[:], t2[:])
                nc.sync.dma_start(ag_in[128 * c : 128 * (c + 1), :], xp[:])
            ag_out = dram.tile([NCORES * D, LC], BF16, addr_space="Shared" if AG_SHARED else "Local")
            nc.gpsimd.collective_compute(
                "AllGather", mybir.AluOpType.bypass, replica_groups=rg,
                ins=[ag_in.opt()], outs=[ag_out.opt()],
            )
            # staircase DMA deferred until after the AllGather is issued
            stair_sb = constp.tile([128, SW], BF16)
            nc.sync.dma_start(stair_sb[:], stair[:, :])
            # gathered xp^T -> 4 SBUF tiles [128, L] (d-chunk major, l = (rank, l'))
            xpT = []
            for c in range(NDCH):
                t = xin.tile([128, L], BF16, tag=f"xpT{c}")
                base = ag_out[:]
                # chunked: 2 ranks per DMA so projections can start early
                for rr in range(0, NCORES, 2):
                    nc.sync.dma_start(
                        t[:, rr * LC : (rr + 2) * LC],
                        _r(base, base.offset + c * 128 * LC + rr * D * LC,
                           [[LC, 128], [D * LC, 2], [1, LC]]),
                    )
                xpT.append(t)

            # ---------------- projections ----------------
            qT = projp.tile([HD, L], BF16, tag="qT")
            kT = projp.tile([HD, L], BF16, tag="kT")
            for wname, bias_sb, dst in (("q", bq_sb, qT), ("k", bk_sb, kT)):
                wt = wsbs[wname]
                for n in range(L // 512):
                    ps = ps_pj.tile([HD, 512], F32, tag="pj")
                    for c in range(NDCH):
                        nc.tensor.matmul(
                            ps[:],
                            wt[:, HD * c : HD * (c + 1)],
                            xpT[c][:, 512 * n : 512 * (n + 1)],
                            start=(c == 0), stop=(c == NDCH - 1),
                        )
                    nc.vector.tensor_scalar_add(
                        dst[:, 512 * n : 512 * (n + 1)], ps[:], bias_sb[:]
                    )
            # token-major V, augmented with a ones column -> [128, 65] per k-block
            vaug = constp.tile([128, 65 * NK], BF16)
            nc.vector.memset(vaug[:, HD::65], 1.0)
            wt = wsbs["v"]
            for lb in range(NK):
                psv = ps_pj.tile([128, HD], F32, tag="pj")
                for c in range(NDCH):
                    nc.tensor.matmul(
                        psv[:],
                        xpT[c][:, 128 * lb : 128 * (lb + 1)],
                        wt[:, HD * c : HD * (c + 1)],
                        start=(c == 0), stop=(c == NDCH - 1),
                    )
                nc.vector.tensor_add(
                    vaug[:, 65 * lb : 65 * lb + HD], psv[:], bvr_sb[:]
                )

            # ---------------- flash attention (transposed layout) ----------------
            oT = projp.tile([HD, L], F32R, tag="oT")  # normalized head output
            rs_in = dram.tile([NCORES, D, QW], BF16)
            rs_out = dram.tile([D, QW], BF16)
            for qc in range(NQ):
                q0 = qc * QW
                pso = ps_op.tile([HD + 1, QW], F32, tag="Oacc")
                for kb in range(NK):
                    k0 = kb * KB
                    pss = ps_sp.tile([KB, QW], F32, tag="s")
                    nc.tensor.matmul(
                        pss[:],
                        kT[:, k0 : k0 + KB],
                        qT[:, q0 : q0 + QW],
                        start=True, stop=True,
                    )
                    st = attnp.tile([KB, QW], BF16, tag="st")
                    nc.scalar.activation(st[:], pss[:], Exp)
                    at = attnp.tile([KB, QW], BF16, tag="at")
                    c0 = 3968 + q0 - k0
                    nc.vector.tensor_mul(
                        at[:], st[:], stair_sb[:, c0 : c0 + QW]
                    )
                    nc.tensor.matmul(
                        pso[:],
                        vaug[:, 65 * kb : 65 * (kb + 1)],
                        at[:],
                        start=(kb == 0), stop=(kb == NK - 1),
                    )
                # normalize: rows 0..63 / row 64
                rec = workp.tile([1, QW], F32R, tag="rec")
                nc.vector.reciprocal(rec[:], pso[HD : HD + 1, :])
                psr = ps_rp.tile([HD, QW], F32, tag="rep")
                nc.tensor.matmul(
                    psr[:], ones_sb[:], rec[:],
                    start=True, stop=True,
                )
                rep = workp.tile([HD, QW], F32, tag="rep_sb")
                nc.vector.tensor_copy(rep[:], psr[:])
                nc.vector.tensor_mul(oT[:, q0 : q0 + QW], pso[0:HD, :], rep[:])
                # fused partial output projection for this q-chunk
                for pd in range(NDCH):
                    psw = ps_rp.tile([128, QW], F32, tag="wo_ps")
                    nc.tensor.matmul(
                        psw[:], wo_sb[:, 128 * pd : 128 * (pd + 1)],
                        oT[:, q0 : q0 + QW],
                        start=True, stop=True,
                    )
                    wt_sb = workp.tile([128, QW], BF16, tag="wo_sb_t")
                    nc.vector.tensor_copy(wt_sb[:], psw[:])
                    base = rs_in[:]
                    nc.sync.dma_start(
                        _r(base, base.offset + (qc * D + 128 * pd) * QW,
                           [[QW, 128], [1, QW]]),
                        wt_sb[:],
                    )

            # ---------------- ReduceScatter + bias ----------------
            nc.gpsimd.collective_compute(
                "ReduceScatter", mybir.AluOpType.add, replica_groups=rg,
                ins=[rs_in.opt()], outs=[rs_out.opt()],
            )
            for pd in range(NDCH):
                t = workp.tile([128, LC], BF16, tag="rs_sb")
                nc.sync.dma_start(t[:], rs_out[128 * pd : 128 * (pd + 1), :])
                ot = workp.tile([128, LC], F32, tag="ot")
                nc.vector.tensor_scalar_add(ot[:], t[:], bo_sb[:, pd : pd + 1])
                nc.sync.dma_start(out[128 * pd : 128 * (pd + 1), :], ot[:])
    return nc


def make_in_maps(x, pos_embed, rel_bias, Wq, bq, Wk, bk, Wv, bv, Wo, bo):
    """Host-side sharding: returns per-core input dicts."""
    x = np.asarray(x, np.float32)
    pos = np.asarray(pos_embed, np.float32)
    rel = np.asarray(rel_bias, np.float32)
    import ml_dtypes
    # exp-staircase per head: stair[p, c] = exp(rel[h, 8063 + p - c]) in bf16
    idx = 8063 + np.arange(128)[:, None] - np.arange(SW)[None, :]
    in_maps = []
    for h in range(NCORES):
        chunk = slice(LC * h, LC * (h + 1))
        in_maps.append({
            "xT": np.ascontiguousarray(x[0, chunk, :].T).astype(ml_dtypes.bfloat16),
            "posT": np.ascontiguousarray(pos[chunk, :].T).astype(ml_dtypes.bfloat16),
            "stair": np.ascontiguousarray(np.exp(rel[h][idx])).astype(ml_dtypes.bfloat16),
            "wq": np.ascontiguousarray(Wq[:, h, :] / 8.0).astype(ml_dtypes.bfloat16),
            "wk": np.ascontiguousarray(Wk[:, h, :]).astype(ml_dtypes.bfloat16),
            "wv": np.ascontiguousarray(Wv[:, h, :]).astype(ml_dtypes.bfloat16),
            "bq": np.ascontiguousarray(bq[h][:, None] / 8.0),
            "bk": np.ascontiguousarray(bk[h][:, None]),
            "bvr": np.ascontiguousarray(np.broadcast_to(bv[h], (128, HD))),
            "wo": np.ascontiguousarray(Wo[h]),
            "bo": np.ascontiguousarray(bo[:, None]),
        })
    return in_maps


_CACHE = {}


def _get_runner():
    """Build + finalize once; return a cached callable in_maps -> results."""
    if "run" in _CACHE:
        return _CACHE["run"]
    nc = build()
    nc.finalize()
    from concourse import bass_utils

    def run(in_maps):
        return bass_utils.run_bass_kernel_spmd(
            nc, in_maps, core_ids=list(range(NCORES))
        ).results

    _CACHE["run"] = run
    return run


def kernel(x, pos_embed, rel_bias, Wq, bq, Wk, bk, Wv, bv, Wo, bo):
    in_maps = make_in_maps(x, pos_embed, rel_bias, Wq, bq, Wk, bk, Wv, bv, Wo, bo)
    results = _get_runner()(in_maps)
    y = np.empty((B, L, D), np.float32)
    for c in range(NCORES):
        y[0, LC * c : LC * (c + 1), :] = results[c]["out"].T
    return y


# revision 18
# speedup vs baseline: 5.5509x; 2.6100x over previous
"""Distributed Trainium2 kernel for relative-position-bias multi-head attention.

Problem: B=1, L=4096, D=512, H=8, HD=64.
    x = x + pos_embed
    q,k,v = x @ W{q,k,v} + b{q,k,v}   (per head)
    scores = (q/8) @ k^T + rel_bias_toeplitz
    out = softmax(scores) @ v ; out = out @ Wo + bo

Sharding: head-parallel. Core h owns head h.
  1. Each core transposes+adds its L/8 chunk of (x, pos) -> xp^T chunk [D, 512],
     AllGather -> full xp^T [D, L] on every core.
  2. Q^T,K^T [64, L] and token-major V [L, 64] for head h (f32r matmuls).
  3. Flash over score tiles in TRANSPOSED layout scores^T [k-part 128, q-free 512]:
     matmul(K^T-slice as weights, Q^T as moving) -> +staircase bias (DVE) ->
     exp (ACT, no max-subtraction; scores ~N(0,1)) -> accumulate
     O^T_unnorm [65, q] via matmul with augmented weights [V | ones].
     Row 64 = softmax denominator. Normalize via reciprocal + ones-outer-product
     partition replication.
  4. AllToAll redistributes head outputs to sequence-parallel rows; each core
     computes its own 512 rows of the output projection. Host transposes and
     concatenates the per-core [D, 512] outputs.

The rel-bias Toeplitz staircase (bias[i,j] = rel[h, L-1+j-i]) is materialized
host-side as one [128, 8064] array per head; every (k-block, q-chunk) bias tile
is a plain column slice of it.
"""
import sys
sys.path.insert(0, '/opt/trn_rl_repo')
import dataclasses

import numpy as np

import concourse.bass as bass
import concourse.tile as tile
from concourse import bacc, mybir

B, L, D, H = 1, 4096, 512, 8
HD = D // H            # 64
NCORES = 8
LC = L // NCORES       # 512 sequence rows per core
NDCH = D // 128        # 4 contraction chunks
QW = 512               # q-chunk width (free dim of score tiles)
NQ = L // QW           # 8
KB = 128               # k-block (partition dim of score tiles)
NK = L // KB           # 32
SW = 8064              # staircase width: col c0 = 3968 + q0 - k0, + 512
AG_SHARED = True       # probes flip this when stubbing the AllGather
F32 = mybir.dt.float32
F32R = mybir.dt.float32r
BF16 = mybir.dt.bfloat16


def _r(ap, offset, pattern):
    return dataclasses.replace(ap, offset=offset, ap=pattern)


def build():
    nc = bacc.Bacc(None, target_bir_lowering=False)

    xT = nc.declare_dram_parameter("xT", [D, LC], BF16, isOutput=False)
    posT = nc.declare_dram_parameter("posT", [D, LC], BF16, isOutput=False)
    stair = nc.declare_dram_parameter("stair", [128, SW], BF16, isOutput=False)
    wq = nc.declare_dram_parameter("wq", [D, HD], BF16, isOutput=False)
    wk = nc.declare_dram_parameter("wk", [D, HD], BF16, isOutput=False)
    wv = nc.declare_dram_parameter("wv", [D, HD], BF16, isOutput=False)
    bq = nc.declare_dram_parameter("bq", [HD, 1], F32, isOutput=False)
    bk = nc.declare_dram_parameter("bk", [HD, 1], F32, isOutput=False)
    bvr = nc.declare_dram_parameter("bvr", [128, HD], F32, isOutput=False)
    wo = nc.declare_dram_parameter("wo", [HD, D], F32R, isOutput=False)
    bo = nc.declare_dram_parameter("bo", [D, 1], F32, isOutput=False)
    out = nc.declare_dram_parameter("out", [D, LC], F32, isOutput=True)

    rg = [list(range(NCORES))]
    Exp = mybir.ActivationFunctionType.Exp

    with tile.TileContext(nc) as tc:
        with (
            nc.allow_low_precision(reason="fp32r matmuls; tolerance 2e-2"),
            tc.tile_pool(name="const", bufs=1) as constp,
            tc.tile_pool(name="xin", bufs=1) as xin,
            tc.tile_pool(name="proj", bufs=1) as projp,
            tc.tile_pool(name="ps_pj", bufs=2, space="PSUM") as ps_pj,
            tc.tile_pool(name="ps_s", bufs=2, space="PSUM") as ps_sp,
            tc.tile_pool(name="ps_o", bufs=2, space="PSUM") as ps_op,
            tc.tile_pool(name="ps_r", bufs=1, space="PSUM") as ps_rp,
            tc.tile_pool(name="attn", bufs=3) as attnp,
            tc.tile_pool(name="work", bufs=2) as workp,
            tc.tile_pool(name="dram", bufs=1, space="DRAM") as dram,
        ):
            # ---------------- constants / weights into SBUF ----------------
            # w{q,k,v} [D, HD] -> [128, NDCH*HD]; chunk c in cols [HD*c, HD*(c+1))
            wsbs = {}
            for name, w in (("q", wq), ("k", wk), ("v", wv)):
                t = constp.tile([128, NDCH * HD], BF16)
                nc.sync.dma_start(
                    t[:], _r(w.ap(), 0, [[HD, 128], [128 * HD, NDCH], [1, HD]])
                )
                wsbs[name] = t
            # wo (head h block) [hd, Dd] -> [64, D]; lhsT slice [:, 128*pd : ...]
            wo_sb = constp.tile([HD, D], F32R)
            nc.sync.dma_start(wo_sb[:], wo[:, :])
            bq_sb = constp.tile([HD, 1], F32)
            nc.sync.dma_start(bq_sb[:], bq[:, :])
            bk_sb = constp.tile([HD, 1], F32)
            nc.sync.dma_start(bk_sb[:], bk[:, :])
            bvr_sb = constp.tile([128, HD], F32)
            nc.sync.dma_start(bvr_sb[:], bvr[:, :])
            bo_sb = constp.tile([128, NDCH], F32)  # chunk pd in col pd
            nc.sync.dma_start(
                bo_sb[:], _r(bo.ap(), 0, [[1, 128], [128, NDCH]])
            )
            ones_f32 = constp.tile([1, HD], F32)
            nc.vector.memset(ones_f32[:], 1.0)
            ones_sb = constp.tile([1, HD], F32R)
            nc.vector.tensor_copy(ones_sb[:], ones_f32[:])

            # ---------------- xp^T chunk + AllGather ----------------
            ag_in = dram.tile([D, LC], BF16)
            for c in range(NDCH):
                t1 = xin.tile([128, LC], BF16, tag="xt")
                nc.sync.dma_start(t1[:], xT[128 * c : 128 * (c + 1), :])
                t2 = xin.tile([128, LC], BF16, tag="pos")
                nc.sync.dma_start(t2[:], posT[128 * c : 128 * (c + 1), :])
                xp = xin.tile([128, LC], BF16, tag="xp")
                nc.vector.tensor_add(xp[:], t1[:], t2[:])
                nc.sync.dma_start(ag_in[128 * c : 128 * (c + 1), :], xp[:])
            ag_out = dram.tile([NCORES * D, LC], BF16, addr_space="Shared" if AG_SHARED else "Local")
            nc.gpsimd.collective_compute(
                "AllGather", mybir.AluOpType.bypass, replica_groups=rg,
                ins=[ag_in.opt()], outs=[ag_out.opt()],
            )
            # staircase DMA deferred until after the AllGather is issued
            stair_sb = constp.tile([128, SW], BF16)
            nc.sync.dma_start(stair_sb[:], stair[:, :])
            # gathered xp^T -> 4 SBUF tiles [128, L] (d-chunk major, l = (rank, l'))
            xpT = []
            for c in range(NDCH):
                t = xin.tile([128, L], BF16, tag=f"xpT{c}")
                base = ag_out[:]
                # chunked: 2 ranks per DMA so projections can start early
                for rr in range(0, NCORES, 2):
                    nc.sync.dma_start(
                        t[:, rr * LC : (rr + 2) * LC],
                        _r(base, base.offset + c * 128 * LC + rr * D * LC,
                           [[LC, 128], [D * LC, 2], [1, LC]]),
                    )
                xpT.append(t)

            # ---------------- projections ----------------
            qT = projp.tile([HD, L], BF16, tag="qT")
            kT = projp.tile([HD, L], BF16, tag="kT")
            for wname, bias_sb, dst in (("q", bq_sb, qT), ("k", bk_sb, kT)):
                wt = wsbs[wname]
                for n in range(L // 512):
                    ps = ps_pj.tile([HD, 512], F32, tag="pj")
                    for c in range(NDCH):
                        nc.tensor.matmul(
                            ps[:],
                            wt[:, HD * c : HD * (c + 1)],
                            xpT[c][:, 512 * n : 512 * (n + 1)],
                            start=(c == 0), stop=(c == NDCH - 1),
                        )
                    nc.vector.tensor_scalar_add(
                        dst[:, 512 * n : 512 * (n + 1)], ps[:], bias_sb[:]
                    )
            # token-major V, augmented with a ones column -> [128, 65] per k-block
            vaug = constp.tile([128, 65 * NK], BF16)
            nc.vector.memset(vaug[:, HD::65], 1.0)
            wt = wsbs["v"]
            for lb in range(NK):
                psv = ps_pj.tile([128, HD], F32, tag="pj")
                for c in range(NDCH):
                    nc.tensor.matmul(
                        psv[:],
                        xpT[c][:, 128 * lb : 128 * (lb + 1)],
                        wt[:, HD * c : HD * (c + 1)],
                        start=(c == 0), stop=(c == NDCH - 1),
                    )
                nc.vector.tensor_add(
                    vaug[:, 65 * lb : 65 * lb + HD], psv[:], bvr_sb[:]
                )

            # ---------------- flash attention (transposed layout) ----------------
            oT = projp.tile([HD, L], F32R, tag="oT")  # normalized head output
            rs_in = dram.tile([NCORES, D, QW], BF16)
            rs_out = dram.tile([D, QW], BF16)
            for qc in range(NQ):
                q0 = qc * QW
                pso = ps_op.tile([HD + 1, QW], F32, tag="Oacc")
                for kb in range(NK):
                    k0 = kb * KB
                    pss = ps_sp.tile([KB, QW], F32, tag="s")
                    nc.tensor.matmul(
                        pss[:],
                        kT[:, k0 : k0 + KB],
                        qT[:, q0 : q0 + QW],
                        start=True, stop=True,
                    )
                    st = attnp.tile([KB, QW], BF16, tag="st")
                    nc.scalar.activation(st[:], pss[:], Exp)
                    at = attnp.tile([KB, QW], BF16, tag="at")
                    c0 = 3968 + q0 - k0
                    nc.vector.tensor_mul(
                        at[:], st[:], stair_sb[:, c0 : c0 + QW]
                    )
                    nc.tensor.matmul(
                        pso[:],
                        vaug[:, 65 * kb : 65 * (kb + 1)],
                        at[:],
                        start=(kb == 0), stop=(kb == NK - 1),
                    )
                # normalize: rows 0..63 / row 64
                rec = workp.tile([1, QW], F32R, tag="rec")
                nc.vector.reciprocal(rec[:], pso[HD : HD + 1, :])
                psr = ps_rp.tile([HD, QW], F32, tag="rep")
                nc.tensor.matmul(
                    psr[:], ones_sb[:], rec[:],
                    start=True, stop=True,
                )
                rep = workp.tile([HD, QW], F32, tag="rep_sb")
                nc.vector.tensor_copy(rep[:], psr[:])
                nc.vector.tensor_mul(oT[:, q0 : q0 + QW], pso[0:HD, :], rep[:])
                # fused partial output projection for this q-chunk
                for pd in range(NDCH):
                    psw = ps_rp.tile([128, QW], F32, tag="wo_ps")
                    nc.tensor.matmul(
                        psw[:], wo_sb[:, 128 * pd : 128 * (pd + 1)],
                        oT[:, q0 : q0 + QW],
                        start=True, stop=True,
                    )
                    wt_sb = workp.tile([128, QW], BF16, tag="wo_sb_t")
                    nc.vector.tensor_copy(wt_sb[:], psw[:])
                    base = rs_in[:]
                    nc.sync.dma_start(
                        _r(base, base.offset + (qc * D + 128 * pd) * QW,
                           [[QW, 128], [1, QW]]),
                        wt_sb[:],
                    )

            # ---------------- ReduceScatter + bias ----------------
            nc.gpsimd.collective_compute(
                "ReduceScatter", mybir.AluOpType.add, replica_groups=rg,
                ins=[rs_in.opt()], outs=[rs_out.opt()],
            )
            for pd in range(NDCH):
                t = workp.tile([128, LC], BF16, tag="rs_sb")
                nc.sync.dma_start(t[:], rs_out[128 * pd : 128 * (pd + 1), :])
                ot = workp.tile([128, LC], F32, tag="ot")
                nc.vector.tensor_scalar_add(ot[:], t[:], bo_sb[:, pd : pd + 1])
                nc.sync.dma_start(out[128 * pd : 128 * (pd + 1), :], ot[:])
    return nc


def make_in_maps(x, pos_embed, rel_bias, Wq, bq, Wk, bk, Wv, bv, Wo, bo):
    """Host-side sharding: returns per-core input dicts."""
    x = np.asarray(x, np.float32)
    pos = np.asarray(pos_embed, np.float32)
    rel = np.asarray(rel_bias, np.float32)
    Wq = np.asarray(Wq, np.float32); bq = np.asarray(bq, np.float32)
    Wk = np.asarray(Wk, np.float32); bk = np.asarray(bk, np.float32)
    Wv = np.asarray(Wv, np.float32); bv = np.asarray(bv, np.float32)
    Wo = np.asarray(Wo, np.float32); bo = np.asarray(bo, np.float32)
    import ml_dtypes
    # exp-staircase per head: stair[p, c] = exp(rel[h, 8063 + p - c]) in bf16
    idx = 8063 + np.arange(128)[:, None] - np.arange(SW)[None, :]
    in_maps = []
    for h in range(NCORES):
        chunk = slice(LC * h, LC * (h + 1))
        in_maps.append({
            "xT": np.ascontiguousarray(x[0, chunk, :].T).astype(ml_dtypes.bfloat16),
            "posT": np.ascontiguousarray(pos[chunk, :].T).astype(ml_dtypes.bfloat16),
            "stair": np.ascontiguousarray(np.exp(rel[h][idx])).astype(ml_dtypes.bfloat16),
            "wq": np.ascontiguousarray(Wq[:, h, :] / 8.0).astype(ml_dtypes.bfloat16),
            "wk": np.ascontiguousarray(Wk[:, h, :]).astype(ml_dtypes.bfloat16),
            "wv": np.ascontiguousarray(Wv[:, h, :]).astype(ml_dtypes.bfloat16),
            "bq": np.ascontiguousarray(bq[h][:, None] / 8.0),
            "bk": np.ascontiguousarray(bk[h][:, None]),
            "bvr": np.ascontiguousarray(np.broadcast_to(bv[h], (128, HD))),
            "wo": np.ascontiguousarray(Wo[h]),
            "bo": np.ascontiguousarray(bo[:, None]),
        })
    return in_maps


_CACHE = {}


def _get_runner():
    """Build + finalize once; return a cached callable in_maps -> results."""
    if "run" in _CACHE:
        return _CACHE["run"]
    nc = build()
    nc.finalize()
    from concourse import bass_utils

    def run(in_maps):
        return bass_utils.run_bass_kernel_spmd(
            nc, in_maps, core_ids=list(range(NCORES))
        ).results

    _CACHE["run"] = run
    return run


def kernel(x, pos_embed, rel_bias, Wq, bq, Wk, bk, Wv, bv, Wo, bo):
    in_maps = make_in_maps(x, pos_embed, rel_bias, Wq, bq, Wk, bk, Wv, bv, Wo, bo)
    results = _get_runner()(in_maps)
    y = np.empty((B, L, D), np.float32)
    for c in range(NCORES):
        y[0, LC * c : LC * (c + 1), :] = results[c]["out"].T
    return y


# revision 32
# speedup vs baseline: 15.5091x; 2.7940x over previous
"""Distributed Trainium2 kernel for relative-position-bias multi-head attention.

Problem: B=1, L=4096, D=512, H=8, HD=64.
    x = x + pos_embed
    q,k,v = x @ W{q,k,v} + b{q,k,v}   (per head)
    scores = (q/8) @ k^T + rel_bias_toeplitz
    out = softmax(scores) @ v ; out = out @ Wo + bo

Sharding: head-parallel. Core h owns head h.
  1. Each core transposes+adds its L/8 chunk of (x, pos) -> xp^T chunk [D, 512],
     AllGather -> full xp^T [D, L] on every core.
  2. Q^T,K^T [64, L] and token-major V [L, 64] for head h (f32r matmuls).
  3. Flash over score tiles in TRANSPOSED layout scores^T [k-part 128, q-free 512]:
     matmul(K^T-slice as weights, Q^T as moving) -> +staircase bias (DVE) ->
     exp (ACT, no max-subtraction; scores ~N(0,1)) -> accumulate
     O^T_unnorm [65, q] via matmul with augmented weights [V | ones].
     Row 64 = softmax denominator. Normalize via reciprocal + ones-outer-product
     partition replication.
  4. AllToAll redistributes head outputs to sequence-parallel rows; each core
     computes its own 512 rows of the output projection. Host transposes and
     concatenates the per-core [D, 512] outputs.

The rel-bias Toeplitz staircase (bias[i,j] = rel[h, L-1+j-i]) is materialized
host-side as one [128, 8064] array per head; every (k-block, q-chunk) bias tile
is a plain column slice of it.
"""
import sys
sys.path.insert(0, '/opt/trn_rl_repo')
import dataclasses

import numpy as np

import concourse.bass as bass
import concourse.tile as tile
from concourse import bacc, mybir

B, L, D, H = 1, 4096, 512, 8
HD = D // H            # 64
NCORES = 8
LC = L // NCORES       # 512 sequence rows per core
NDCH = D // 128        # 4 contraction chunks
QW = 512               # q-chunk width (free dim of score tiles)
NQ = L // QW           # 8
KB = 128               # k-block (partition dim of score tiles)
NK = L // KB           # 32
SW = 8064              # staircase width: col c0 = 3968 + q0 - k0, + 512
AG_SHARED = True       # probes flip this when stubbing the AllGather
F32 = mybir.dt.float32
F32R = mybir.dt.float32r
BF16 = mybir.dt.bfloat16


def _r(ap, offset, pattern):
    return dataclasses.replace(ap, offset=offset, ap=pattern)


def build():
    nc = bacc.Bacc(None, target_bir_lowering=False)

    xT = nc.declare_dram_parameter("xT", [D, LC], BF16, isOutput=False)
    posT = nc.declare_dram_parameter("posT", [D, LC], BF16, isOutput=False)
    stair = nc.declare_dram_parameter("stair", [128, SW], BF16, isOutput=False)
    wq = nc.declare_dram_parameter("wq", [D, HD], BF16, isOutput=False)
    wk = nc.declare_dram_parameter("wk", [D, HD], BF16, isOutput=False)
    wv = nc.declare_dram_parameter("wv", [D, HD], BF16, isOutput=False)
    bq = nc.declare_dram_parameter("bq", [HD, 1], F32, isOutput=False)
    bk = nc.declare_dram_parameter("bk", [HD, 1], F32, isOutput=False)
    bvr = nc.declare_dram_parameter("bvr", [128, HD], F32, isOutput=False)
    wo = nc.declare_dram_parameter("wo", [HD, D], F32R, isOutput=False)
    bo = nc.declare_dram_parameter("bo", [D, 1], F32, isOutput=False)
    out = nc.declare_dram_parameter("out", [D, LC], F32, isOutput=True)

    rg = [list(range(NCORES))]
    Exp = mybir.ActivationFunctionType.Exp

    with tile.TileContext(nc) as tc:
        with (
            nc.allow_low_precision(reason="fp32r matmuls; tolerance 2e-2"),
            tc.tile_pool(name="const", bufs=1) as constp,
            tc.tile_pool(name="xin", bufs=2) as xin,
            tc.tile_pool(name="proj", bufs=1) as projp,
            tc.tile_pool(name="ps_pj", bufs=1, space="PSUM") as ps_pj,
            tc.tile_pool(name="ps_s", bufs=2, space="PSUM") as ps_sp,
            tc.tile_pool(name="ps_o", bufs=2, space="PSUM") as ps_op,
            tc.tile_pool(name="ps_r", bufs=1, space="PSUM") as ps_rp,
            tc.tile_pool(name="attn", bufs=4) as attnp,
            tc.tile_pool(name="work", bufs=2) as workp,
            tc.tile_pool(name="dram", bufs=1, space="DRAM") as dram,
        ):
            # ---------------- constants / weights into SBUF ----------------
            # w{q,k,v} [D, HD] -> [128, NDCH*HD]; chunk c in cols [HD*c, HD*(c+1))
            wsbs = {}
            for name, w in (("q", wq), ("k", wk), ("v", wv)):
                t = constp.tile([128, NDCH * HD], BF16, tag=f"w_{name}")
                nc.gpsimd.dma_start(
                    t[:], _r(w.ap(), 0, [[HD, 128], [128 * HD, NDCH], [1, HD]])
                )
                wsbs[name] = t
            # wo (head h block) [hd, Dd] -> [64, D]; lhsT slice [:, 128*pd : ...]
            wo_sb = constp.tile([HD, D], F32R)
            nc.gpsimd.dma_start(wo_sb[:], wo[:, :])
            bq_sb = constp.tile([HD, 1], F32)
            nc.gpsimd.dma_start(bq_sb[:], bq[:, :])
            bk_sb = constp.tile([HD, 1], F32)
            nc.gpsimd.dma_start(bk_sb[:], bk[:, :])
            bvr_sb = constp.tile([128, HD], F32)
            nc.gpsimd.dma_start(bvr_sb[:], bvr[:, :])
            bo_sb = constp.tile([128, NDCH], F32)  # chunk pd in col pd
            nc.gpsimd.dma_start(
                bo_sb[:], _r(bo.ap(), 0, [[1, 128], [128, NDCH]])
            )
            ones_f32 = constp.tile([1, HD], F32)
            nc.vector.memset(ones_f32[:], 1.0)
            ones_sb = constp.tile([1, HD], F32R)
            nc.vector.tensor_copy(ones_sb[:], ones_f32[:])

            # ---------------- xp^T chunk + AllGather ----------------
            ag_in = dram.tile([D, LC], BF16)
            for c in range(NDCH):
                t1 = xin.tile([128, LC], BF16, tag="xt")
                nc.sync.dma_start(t1[:], xT[128 * c : 128 * (c + 1), :])
                t2 = xin.tile([128, LC], BF16, tag="pos")
                nc.gpsimd.dma_start(t2[:], posT[128 * c : 128 * (c + 1), :])
                xp = xin.tile([128, LC], BF16, tag="xp")
                nc.vector.tensor_add(xp[:], t1[:], t2[:])
                nc.sync.dma_start(ag_in[128 * c : 128 * (c + 1), :], xp[:])
            ag_out = dram.tile([NCORES * D, LC], BF16, addr_space="Shared" if AG_SHARED else "Local")
            nc.gpsimd.collective_compute(
                "AllGather", mybir.AluOpType.bypass, replica_groups=rg,
                ins=[ag_in.opt()], outs=[ag_out.opt()],
            )
            # staircase DMA deferred until after the AllGather is issued
            stair_sb = constp.tile([128, SW], BF16)
            nc.sync.dma_start(stair_sb[:], stair[:, :])
            # gathered xp^T -> 4 SBUF tiles [128, L] (d-chunk major, l = (rank, l'))
            xpT = []
            for c in range(NDCH):
                t = xin.tile([128, L], BF16, tag=f"xpT{c}")
                base = ag_out[:]
                # chunked: 2 ranks per DMA so projections can start early
                for rr in range(0, NCORES, 2):
                    eng = nc.sync if (rr // 2) % 2 == 0 else nc.gpsimd
                    eng.dma_start(
                        t[:, rr * LC : (rr + 2) * LC],
                        _r(base, base.offset + c * 128 * LC + rr * D * LC,
                           [[LC, 128], [D * LC, 2], [1, LC]]),
                    )
                xpT.append(t)

            # ---------------- projections ----------------
            qT = projp.tile([HD, L], BF16, tag="qT")
            kT = projp.tile([HD, L], BF16, tag="kT")
            # token-major V, augmented with a ones column -> [128, 65] per k-block
            vaug = constp.tile([128, 65 * NK], BF16)
            nc.vector.memset(vaug[:, HD::65], 1.0)
            # interleaved per n-chunk (k, q, then the V l-blocks of that chunk)
            # so flash q-chunk 0 unblocks after n=0; separate PSUM tags per
            # tile shape (one shared tag deadlocks the scheduling pass).
            for n in range(L // 512):
                for wname, bias_sb, dst in (("k", bk_sb, kT), ("q", bq_sb, qT)):
                    wt = wsbs[wname]
                    ps = ps_pj.tile([HD, 512], F32, tag="pj_qk")
                    for c in range(NDCH):
                        nc.tensor.matmul(
                            ps[:],
                            wt[:, HD * c : HD * (c + 1)],
                            xpT[c][:, 512 * n : 512 * (n + 1)],
                            start=(c == 0), stop=(c == NDCH - 1),
                        )
                    nc.vector.tensor_scalar_add(
                        dst[:, 512 * n : 512 * (n + 1)], ps[:], bias_sb[:]
                    )
                wt = wsbs["v"]
                for lb in range(4 * n, 4 * n + 4):
                    psv = ps_pj.tile([128, HD], F32, tag="pj_v")
                    for c in range(NDCH):
                        nc.tensor.matmul(
                            psv[:],
                            xpT[c][:, 128 * lb : 128 * (lb + 1)],
                            wt[:, HD * c : HD * (c + 1)],
                            start=(c == 0), stop=(c == NDCH - 1),
                        )
                    nc.vector.tensor_add(
                        vaug[:, 65 * lb : 65 * lb + HD], psv[:], bvr_sb[:]
                    )
            # ---------------- flash attention (transposed layout) ----------------
            oT = projp.tile([HD, L], F32R, tag="oT")  # normalized head output
            rs_in = dram.tile([NCORES, D, QW], BF16)
            rs_out = dram.tile([D, QW], BF16)
            for qc in range(NQ):
                q0 = qc * QW
                pso = ps_op.tile([HD + 1, QW], F32, tag="Oacc")
                for kb in range(NK):
                    k0 = kb * KB
                    pss = ps_sp.tile([KB, QW], F32, tag="s")
                    nc.tensor.matmul(
                        pss[:],
                        kT[:, k0 : k0 + KB],
                        qT[:, q0 : q0 + QW],
                        start=True, stop=True,
                    )
                    st = attnp.tile([KB, QW], BF16, tag="st")
                    nc.scalar.activation(st[:], pss[:], Exp)
                    at = attnp.tile([KB, QW], BF16, tag="at")
                    c0 = 3968 + q0 - k0
                    nc.vector.tensor_mul(
                        at[:], st[:], stair_sb[:, c0 : c0 + QW]
                    )
                    nc.tensor.matmul(
                        pso[:],
                        vaug[:, 65 * kb : 65 * (kb + 1)],
                        at[:],
                        start=(kb == 0), stop=(kb == NK - 1),
                    )
                # normalize: rows 0..63 / row 64
                rec = workp.tile([1, QW], F32R, tag="rec")
                nc.vector.reciprocal(rec[:], pso[HD : HD + 1, :])
                psr = ps_rp.tile([HD, QW], F32, tag="rw")
                nc.tensor.matmul(
                    psr[:], ones_sb[:], rec[:],
                    start=True, stop=True,
                )
                rep = workp.tile([HD, QW], F32R, tag="rep_sb")
                nc.vector.tensor_copy(rep[:], psr[:])
                nc.vector.tensor_mul(oT[:, q0 : q0 + QW], pso[0:HD, :], rep[:])
                # fused partial output projection for this q-chunk
                for pd in range(NDCH):
                    psw = ps_rp.tile([128, QW], F32, tag="rw")
                    nc.tensor.matmul(
                        psw[:], wo_sb[:, 128 * pd : 128 * (pd + 1)],
                        oT[:, q0 : q0 + QW],
                        start=True, stop=True,
                    )
                    wt_sb = workp.tile([128, QW], BF16, tag="wo_sb_t")
                    nc.vector.tensor_copy(wt_sb[:], psw[:])
                    base = rs_in[:]
                    nc.sync.dma_start(
                        _r(base, base.offset + (qc * D + 128 * pd) * QW,
                           [[QW, 128], [1, QW]]),
                        wt_sb[:],
                    )

            # ---------------- ReduceScatter + bias ----------------
            nc.gpsimd.collective_compute(
                "ReduceScatter", mybir.AluOpType.add, replica_groups=rg,
                ins=[rs_in.opt()], outs=[rs_out.opt()],
            )
            for pd in range(NDCH):
                t = workp.tile([128, LC], BF16, tag="rs_sb")
                nc.sync.dma_start(t[:], rs_out[128 * pd : 128 * (pd + 1), :])
                ot = workp.tile([128, LC], F32, tag="ot")
                nc.vector.tensor_scalar_add(ot[:], t[:], bo_sb[:, pd : pd + 1])
                nc.sync.dma_start(out[128 * pd : 128 * (pd + 1), :], ot[:])
    return nc


def make_in_maps(x, pos_embed, rel_bias, Wq, bq, Wk, bk, Wv, bv, Wo, bo):
    """Host-side sharding: returns per-core input dicts."""
    x = np.asarray(x, np.float32)
    pos = np.asarray(pos_embed, np.float32)
    rel = np.asarray(rel_bias, np.float32)
    Wq = np.asarray(Wq, np.float32); bq = np.asarray(bq, np.float32)
    Wk = np.asarray(Wk, np.float32); bk = np.asarray(bk, np.float32)
    Wv = np.asarray(Wv, np.float32); bv = np.asarray(bv, np.float32)
    Wo = np.asarray(Wo, np.float32); bo = np.asarray(bo, np.float32)
    import ml_dtypes
    # exp-staircase per head: stair[p, c] = exp(rel[h, 8063 + p - c]) in bf16
    idx = 8063 + np.arange(128)[:, None] - np.arange(SW)[None, :]
    in_maps = []
    for h in range(NCORES):
        chunk = slice(LC * h, LC * (h + 1))
        in_maps.append({
            "xT": np.ascontiguousarray(x[0, chunk, :].T).astype(ml_dtypes.bfloat16),
            "posT": np.ascontiguousarray(pos[chunk, :].T).astype(ml_dtypes.bfloat16),
            "stair": np.ascontiguousarray(np.exp(rel[h][idx])).astype(ml_dtypes.bfloat16),
            "wq": np.ascontiguousarray(Wq[:, h, :] / 8.0).astype(ml_dtypes.bfloat16),
            "wk": np.ascontiguousarray(Wk[:, h, :]).astype(ml_dtypes.bfloat16),
            "wv": np.ascontiguousarray(Wv[:, h, :]).astype(ml_dtypes.bfloat16),
            "bq": np.ascontiguousarray(bq[h][:, None] / 8.0),
            "bk": np.ascontiguousarray(bk[h][:, None]),
            "bvr": np.ascontiguousarray(np.broadcast_to(bv[h], (128, HD))),
            "wo": np.ascontiguousarray(Wo[h]),
            "bo": np.ascontiguousarray(bo[:, None]),
        })
    return in_maps


_CACHE = {}


def _get_runner():
    """Build + finalize once; return a cached callable in_maps -> results."""
    if "run" in _CACHE:
        return _CACHE["run"]
    nc = build()
    nc.finalize()
    from concourse import bass_utils

    def run(in_maps):
        return bass_utils.run_bass_kernel_spmd(
            nc, in_maps, core_ids=list(range(NCORES))
        ).results

    _CACHE["run"] = run
    return run


def kernel(x, pos_embed, rel_bias, Wq, bq, Wk, bk, Wv, bv, Wo, bo):
    in_maps = make_in_maps(x, pos_embed, rel_bias, Wq, bq, Wk, bk, Wv, bv, Wo, bo)
    results = _get_runner()(in_maps)
    y = np.empty((B, L, D), np.float32)
    for c in range(NCORES):
        y[0, LC * c : LC * (c + 1), :] = results[c]["out"].T
    return y
